# revision 14
# baseline (speedup 1.0000x reference)
"""Trainium2 Bass kernel for nn_Attention_51384988730000 (sparse_attention).

Data-parallel over batch: 8 batch elements -> 8 NeuronCores, one element each.
Per-core computation (all matmuls in float32r = fp32 data at full PE rate):

  x [1025,768] --LN--> xn --PE-transpose--> xnT [768,1026] (feature-major,
      token dim padded to 1026 so every matmul free-dim is even, as fp32r
      requires; the pad column is garbage but finite)
  k  = kv_w[:, :768]^T @ xnT           (d-major [768,1026])
  v  = xn @ kv_w[:, 768:]              (token-major, + ones column per head;
      the pad key's ones-entry is 0 so it contributes nothing to attention)
  dw = depthwise 5x5 conv on xnT       (25 diagonal matmuls, PSUM-accumulated)
  q  = pw_w @ dw  (+ cls passthrough)  (d-major [768,1026])
  rotary(q, k): partial, pair-swap via PE permutation matmul + DVE combine
  per head: dotsT = k_h @ q_h^T ; attnT = exp(scale*dotsT) (ACT, no max-sub --
            logits are O(1) for this distribution); avT = v_aug^T @ attnT with
            the ones column producing the softmax denominator.
  normalize via GPSIMD partition-broadcast of 1/den ; out = o @ out_w + b
"""

import os
import numpy as np

import concourse.bass as bass
import concourse.tile as tile
from concourse import bacc, mybir
from concourse import bass_utils

F32R = mybir.dt.float32r
BF16 = mybir.dt.bfloat16
F32 = mybir.dt.float32
AF = mybir.ActivationFunctionType
ALU = mybir.AluOpType

DIM = 768
HEADS = 12
DH = 64
NTOK = 1025
NT2 = 1026          # padded on-chip token dim (even)
NSP = 1024
NCH = 6             # 768 / 128
TCH = 9             # token chunks of 128
KSZ = [128] * 8 + [2]   # key/query padded chunk widths (on-chip)
YSZ = [128] * 8 + [1]   # real token counts (DRAM I/O)
IC2 = [(0, 512), (512, 1024), (1024, 1026)]   # bank-aligned chunks (k, S, AV)
IC3 = [(0, 342), (342, 684), (684, 1026)]     # single-bank dots chunks
SCALE = float(DH) ** -0.5
EPS = 1e-5
NCORES = 8


def _bcast_ap(ap, parts):
    """Partition-broadcast view of a 1D DRAM AP."""
    return bass.AP(tensor=ap.tensor, offset=ap.offset, ap=[[0, parts]] + list(ap.ap))


def _f32(ap):
    return ap.bitcast(F32)


def _build_program():
    nc = bacc.Bacc("TRN2", target_bir_lowering=False, debug=False, num_devices=NCORES)

    dr = {}
    dr["x"] = nc.dram_tensor("x", [NTOK, DIM], F32R, kind="ExternalInput").ap()
    dr["wk"] = nc.dram_tensor("wk", [DIM, DIM], F32R, kind="ExternalInput").ap()
    dr["wv"] = nc.dram_tensor("wv", [DIM, DIM], F32R, kind="ExternalInput").ap()
    dr["wq"] = nc.dram_tensor("wq", [DIM, DIM], F32R, kind="ExternalInput").ap()
    dr["wo"] = nc.dram_tensor("wo", [DIM, DIM], F32R, kind="ExternalInput").ap()
    dr["wd"] = nc.dram_tensor("wd", [DIM, 25], F32R, kind="ExternalInput").ap()
    dr["cosf"] = nc.dram_tensor("cosf", [128, NT2], F32R, kind="ExternalInput").ap()
    dr["sinf"] = nc.dram_tensor("sinf", [128, NT2], F32R, kind="ExternalInput").ap()
    dr["ident"] = nc.dram_tensor("ident", [128, 128], F32R, kind="ExternalInput").ap()
    dr["ones12"] = nc.dram_tensor("ones12", [128, HEADS], F32R, kind="ExternalInput").ap()
    dr["ones12z"] = nc.dram_tensor("ones12z", [128, HEADS], F32R, kind="ExternalInput").ap()
    dr["vzero"] = nc.dram_tensor("vzero", [1, HEADS * (DH + 1)], F32R, kind="ExternalInput").ap()
    dr["gvec"] = nc.dram_tensor("gvec", [DIM], F32, kind="ExternalInput").ap()
    dr["bvec"] = nc.dram_tensor("bvec", [DIM], F32, kind="ExternalInput").ap()
    dr["obias"] = nc.dram_tensor("obias", [DIM], F32, kind="ExternalInput").ap()
    dr["y"] = nc.dram_tensor("y", [NTOK, DIM], F32, kind="ExternalOutput").ap()

    reps = int(os.environ.get("REPLICAS", "1"))
    with tile.TileContext(nc) as tc:
        for _ in range(reps):
            _emit(nc, tc, dr)
    nc.compile()
    return nc


def _emit(nc, tc, dr):
    from contextlib import ExitStack

    ctx = ExitStack()
    with ctx:
        singles = ctx.enter_context(tc.tile_pool(name="singles", bufs=1))
        acts = ctx.enter_context(tc.tile_pool(name="acts", bufs=1))

        ident = singles.tile([128, 128], F32R, tag="ident")
        nc.sync.dma_start(out=ident, in_=dr["ident"])
        cosf = singles.tile([128, NT2], F32R, tag="cosf")
        sinf = singles.tile([128, NT2], F32R, tag="sinf")
        g_sb = singles.tile([128, NCH], F32, tag="g")
        nc.sync.dma_start(out=g_sb, in_=dr["gvec"].rearrange("(c p) -> p c", p=128))
        b_sb = singles.tile([128, NCH], F32, tag="b")
        nc.sync.dma_start(out=b_sb, in_=dr["bvec"].rearrange("(c p) -> p c", p=128))
        ob_sb = singles.tile([128, DIM], F32, tag="ob")
        cls_sb = singles.tile([128, NCH], F32R, tag="cls")
        wd_sb = singles.tile([128, NCH, 25], F32R, tag="wd")
        eps_sb = singles.tile([128, 1], F32, tag="eps")
        nc.vector.memset(eps_sb, EPS)

        dwp_ctx = tc.tile_pool(name="dwp", bufs=1, side="right")
        dwp = dwp_ctx.__enter__()
        xnTp_ctx = tc.tile_pool(name="xnTp", bufs=1, side="right")
        xnTp = xnTp_ctx.__enter__()
        xnT = [xnTp.tile([128, NT2], F32R, tag=f"xnT{c}", name=f"xnT{c}") for c in range(NCH)]

        # ====== Phase A: load x, layernorm, transpose, v projection ======
        # (v for token chunk t only needs chunk t's freshly transposed
        #  columns, so its matmuls fill PE while DVE runs the next LN)
        krot, qrot, v_sb = [], [], []
        with tc.tile_pool(name="wvp", bufs=1) as wvp, \
             tc.tile_pool(name="xa", bufs=int(os.environ.get("XAB", "4"))) as xpool, \
             tc.tile_pool(name="stats", bufs=int(os.environ.get("STB", "8"))) as spool, \
             tc.tile_pool(name="pst", bufs=int(os.environ.get("PSTB", "4")), space="PSUM") as pst, \
             tc.tile_pool(name="psvp", bufs=int(os.environ.get("PSVB", "2")), space="PSUM") as psvp:
            wv_sb = []
            for c in range(NCH):
                w = wvp.tile([128, DIM], F32R, tag=f"wv{c}", name=f"wv{c}")
                nc.sync.dma_start(out=w, in_=dr["wv"][c * 128:(c + 1) * 128])
                wv_sb.append(w)
            def emit_v(t):
                wid = KSZ[t]
                ps = psvp.tile([128, DIM], F32, tag="psv")
                for c in range(NCH):
                    st_, sp_ = c == 0, c == NCH - 1
                    lhs = xnT[c][:, t * 128:t * 128 + wid]
                    nc.tensor.matmul(ps[:wid, 0:512], lhs, wv_sb[c][:, 0:512], start=st_, stop=sp_)
                    nc.tensor.matmul(ps[:wid, 512:768], lhs, wv_sb[c][:, 512:768], start=st_, stop=sp_)
                v_sb.append(acts.tile([128, HEADS, DH + 1], F32R, tag=f"v{t}", name=f"v{t}"))
                nc.sync.dma_start(out=v_sb[t][:, :, DH:DH + 1],
                                  in_=dr["ones12"].rearrange("p (h o) -> p h o", o=1))
                nc.scalar.copy(v_sb[t][:wid, :, 0:DH],
                               ps[:wid].rearrange("p (h d) -> p h d", d=DH))
                if t == 8:
                    # the padded garbage key must contribute exactly nothing:
                    # zero its whole v_aug row (v values and ones entry)
                    nc.sync.dma_start(
                        out=v_sb[t][1:2, :, :],
                        in_=dr["vzero"].rearrange("p (h d) -> p h d", d=DH + 1))

            for t in range(TCH):
                rows = YSZ[t]
                xt = xpool.tile([128, DIM], F32R, tag="x")
                nc.sync.dma_start(out=xt[:rows], in_=dr["x"][t * 128:t * 128 + rows])
                if t == TCH - 1:
                    # pad token rows: replicate the last real token so the LN /
                    # transpose reads (which span all 128 partitions) never see
                    # stale SBUF (which could be Inf/NaN and poison real
                    # outputs through exp(pad dots) * 0 = NaN)
                    last = dr["x"][t * 128 + rows - 1:t * 128 + rows]
                    nc.sync.dma_start(
                        out=xt[rows:128],
                        in_=bass.AP(tensor=last.tensor, offset=last.offset,
                                    ap=[[0, 128 - rows]] + list(last.ap[1:])))
                st = spool.tile([128, 3, 6], F32, tag="st")
                xg = xt.rearrange("p (s f) -> p s f", f=256)
                for sg in range(3):
                    nc.vector.bn_stats(st[:, sg, :], xg[:, sg, :])
                mv = spool.tile([128, 2], F32, tag="mv")
                nc.vector.bn_aggr(mv, st)
                std = spool.tile([128, 1], F32, tag="std")
                nc.scalar.activation(std, mv[:, 1:2], AF.Sqrt, bias=eps_sb)
                rstd = spool.tile([128, 1], F32, tag="rstd")
                nc.vector.reciprocal(rstd, std)
                nmr = spool.tile([128, 1], F32, tag="nmr")
                nc.vector.tensor_scalar(out=nmr, in0=mv[:, 0:1], scalar1=rstd,
                                        scalar2=-1.0, op0=ALU.mult, op1=ALU.mult)
                xc = xpool.tile([128, DIM], F32R, tag="xc")
                eng = nc.vector if t % 2 == 0 else nc.gpsimd
                eng.tensor_scalar(out=xc, in0=xt, scalar1=rstd, scalar2=nmr,
                                  op0=ALU.mult, op1=ALU.add)
                wid = KSZ[t]
                for c in range(NCH):
                    pt = pst.tile([128, 128], F32R, tag="pt")
                    nc.tensor.transpose(pt, xc[:, c * 128:(c + 1) * 128], ident)
                    dst = xnT[c][:, t * 128:t * 128 + wid]
                    nc.scalar.activation(dst, pt[:, 0:wid], AF.Identity,
                                         bias=b_sb[:, c:c + 1], scale=g_sb[:, c:c + 1])
                # v projection, software-pipelined one tile behind the
                # transposes so PE never waits on the ACT evacuation
                if t >= 1:
                    emit_v(t - 1)
                if t == TCH - 1:
                    emit_v(t)
            for c in range(NCH):
                nc.scalar.copy(cls_sb[:, c:c + 1], xnT[c][:, 0:1])
            # deferred constant loads (keep the startup DMA queue clear for x)
            nc.sync.dma_start(out=cosf, in_=dr["cosf"])
            nc.sync.dma_start(out=sinf, in_=dr["sinf"])
            nc.sync.dma_start(out=ob_sb, in_=_bcast_ap(dr["obias"], 128))
            nc.sync.dma_start(out=wd_sb, in_=dr["wd"].rearrange("(c p) t -> p c t", p=128))

        # ============ Phase B1: k projection (d-major) + rotary ============
        SWAP_MASK = [i ^ 1 for i in range(32)]

        def rotary(tin, tmp_pool, out_tile):
            """out = tin*cosf + pairswap(tin)*sinf ; tin is SBUF [128, NT2].
            stream_shuffle swaps pairs in every 32-partition quadrant; the
            pass-through quadrants are killed by sinf=0 rows."""
            shuf = tmp_pool.tile([128, NT2], F32, tag="shuf")
            nc.vector.stream_shuffle(shuf, _f32(tin), SWAP_MASK)
            tS = tmp_pool.tile([128, NT2], F32, tag="tS")
            nc.vector.tensor_mul(tS, shuf, sinf)
            tC = tmp_pool.tile([128, NT2], F32, tag="tC")
            nc.vector.tensor_mul(tC, tin, cosf)
            nc.gpsimd.tensor_add(out_tile, tC, tS)

        with tc.tile_pool(name="wkp", bufs=1) as wkp, \
             tc.tile_pool(name="pskp", bufs=2, space="PSUM") as pskp, \
             tc.tile_pool(name="ktmp", bufs=1) as ktmp:
            wk_sb = []
            for kc in range(NCH):
                w = wkp.tile([128, DIM], F32R, tag=f"wk{kc}", name=f"wk{kc}")
                nc.sync.dma_start(out=w, in_=dr["wk"][kc * 128:(kc + 1) * 128])
                wk_sb.append(w)
            for c in range(NCH):
                ps = pskp.tile([128, NT2], F32, tag="psk")
                for kc in range(NCH):
                    st, sp = kc == 0, kc == NCH - 1
                    lhs = wk_sb[kc][:, c * 128:(c + 1) * 128]
                    for lo, hi in IC2:
                        nc.tensor.matmul(ps[:, lo:hi], lhs, xnT[kc][:, lo:hi],
                                         start=st, stop=sp)
                kT = ktmp.tile([128, NT2], F32R, tag="kT", bufs=2)
                nc.scalar.copy(kT, ps)
                krot.append(acts.tile([128, NT2], F32R, tag=f"krot{c}", name=f"krot{c}"))
                rotary(kT, ktmp, krot[c])

        # ============ Phase B3: depthwise conv (diag matmuls) ============
        taps = [(0, 0)] + [(dy, dx) for dy in range(-2, 3) for dx in range(-2, 3)
                           if (dy, dx) != (0, 0)]
        # taps with dx == +1 run on DVE (idle in this era) as FMA chains
        # into an SBUF accumulator; the rest stay PE diag matmuls. This also
        # removes those taps' odd-alignment fp32 special cases.
        dve_taps = [(dy, 1) for dy in range(-2, 3)]
        ngp = int(os.environ.get("GPT", "0"))
        gp_taps = [(dy, -1) for dy in range(-2, 3)][:ngp]
        pe_taps = [t for t in taps if t not in dve_taps and t not in gp_taps]
        with tc.tile_pool(name="diag", bufs=int(os.environ.get("DGB", "5"))) as dgp, \
             tc.tile_pool(name="dacc", bufs=2) as daccp, \
             tc.tile_pool(name="psdp", bufs=2, space="PSUM") as psdp:
            dwT = []
            for c in range(NCH):
                ps = psdp.tile([128, NSP], F32, tag="psd")
                psg = ps.rearrange("p (y x) -> p y x", x=32)
                sp_in = xnT[c][:, 1:1025].rearrange("p (y x) -> p y x", x=32)
                acc = daccp.tile([128, NSP], F32, tag="dacc")
                nc.gpsimd.memset(acc, 0.0)
                accg = acc.rearrange("p (y x) -> p y x", x=32)
                for eng, tlist in ((nc.vector, dve_taps), (nc.gpsimd, gp_taps)):
                    for (dy, dx) in tlist:
                        tap_idx = (dy + 2) * 5 + (dx + 2)
                        y_lo, y_hi = max(0, -dy), min(32, 32 - dy)
                        xl, xh = max(0, -dx), min(32, 32 - dx)
                        o_ap = accg[:, y_lo:y_hi, xl:xh]
                        i_ap = sp_in[:, y_lo + dy:y_hi + dy, xl + dx:xh + dx]
                        eng.scalar_tensor_tensor(
                            out=o_ap, in0=_f32(i_ap),
                            scalar=wd_sb[:, c, tap_idx:tap_idx + 1].bitcast(F32),
                            in1=o_ap, op0=ALU.mult, op1=ALU.add)
                for ti, (dy, dx) in enumerate(pe_taps):
                    tap_idx = (dy + 2) * 5 + (dx + 2)
                    dg = dgp.tile([128, 128], F32R, tag="dg")
                    wcol = wd_sb[:, c, tap_idx:tap_idx + 1]
                    # dg = diag(w) = ident * w (per-partition scalar) — much
                    # cheaper than affine_select; alternate engines.
                    deng = nc.vector if ti % 2 == 0 else nc.gpsimd
                    deng.tensor_scalar(out=dg, in0=ident,
                                       scalar1=wcol.bitcast(F32), scalar2=None,
                                       op0=ALU.mult)
                    if dx == 0:
                        xranges = [(0, 32, False)]
                    elif dx == -1:
                        xranges = [(2, 32, False), (1, 2, True)]
                    elif dx == 2:
                        xranges = [(0, 30, False)]
                    else:
                        xranges = [(2, 32, False)]
                    for (xl, xh, f32mm) in xranges:
                        for hh in range(2):
                            y_lo, y_hi = max(hh * 16, -dy), min(hh * 16 + 16, 32 - dy)
                            if y_hi <= y_lo:
                                continue
                            o_ap = psg[:, y_lo:y_hi, xl:xh]
                            i_ap = sp_in[:, y_lo + dy:y_hi + dy, xl + dx:xh + dx]
                            w_ap = dg
                            if f32mm:
                                i_ap, w_ap = _f32(i_ap), _f32(w_ap)
                            nc.tensor.matmul(
                                o_ap, w_ap, i_ap,
                                start=(ti == 0), stop=(ti == len(pe_taps) - 1),
                                skip_group_check=True)
                dt_ = dwp.tile([128, NSP], F32R, tag=f"dwT{c}", name=f"dwT{c}")
                nc.vector.tensor_add(dt_, ps, acc)
                dwT.append(dt_)
        xnTp_ctx.__exit__(None, None, None)

        # ===== Phase B4 + C: pointwise conv -> q -> rotary, interleaved =====
        # with attention head pairs (shares the 2-bank PSUM pool "pbig")
        _SKIP_ATTN = os.environ.get("SKIP_ATTN", "0") == "1"
        SM = SCALE
        wqp_ctx = tc.tile_pool(name="wqp", bufs=1, side="right")
        wqp = wqp_ctx.__enter__()
        qtmp_ctx = tc.tile_pool(name="qtmp", bufs=1, side="right")
        qtmp = qtmp_ctx.__enter__()
        with tc.tile_pool(name="pbig", bufs=int(os.environ.get("PBB", "2")), space="PSUM") as pbig, \
             tc.tile_pool(name="pavp", bufs=int(os.environ.get("AVB", "1")), space="PSUM") as pavp:
            wq_sb = []
            for c in range(NCH):
                w = wqp.tile([128, DIM], F32R, tag=f"wq{c}", name=f"wq{c}")
                nc.sync.dma_start(out=w, in_=dr["wq"][c * 128:(c + 1) * 128])
                wq_sb.append(w)

            def emit_pw(o):
                ps = pbig.tile([128, NSP], F32, tag="pbig")
                for c in range(NCH):
                    st, sp = c == 0, c == NCH - 1
                    lhs = wq_sb[c][:, o * 128:(o + 1) * 128]
                    nc.tensor.matmul(ps[:, 0:512], lhs, dwT[c][:, 0:512], start=st, stop=sp)
                    nc.tensor.matmul(ps[:, 512:1024], lhs, dwT[c][:, 512:1024], start=st, stop=sp)
                qT = qtmp.tile([128, NT2], F32R, tag="qT", bufs=2)
                nc.scalar.copy(qT[:, 0:1], cls_sb[:, o:o + 1])
                nc.scalar.copy(qT[:, 1025:1026], cls_sb[:, o:o + 1])
                nc.scalar.copy(qT[:, 1:1025], ps)
                qrot.append(acts.tile([128, NT2], F32R, tag=f"qrot{o}", name=f"qrot{o}"))
                rotary(qT, qtmp, qrot[o])

            for o in range(NCH):
                emit_pw(o)
            qtmp_ctx.__exit__(None, None, None)
            wqp_ctx.__exit__(None, None, None)
            dwp_ctx.__exit__(None, None, None)

            with tc.tile_pool(name="attnp", bufs=int(os.environ.get("ATB", "8"))) as attnp, \
                 tc.tile_pool(name="asmp", bufs=int(os.environ.get("ASB", "2"))) as asmp:
                oT = [acts.tile([128, NT2], F32R, tag=f"oT{c}", name=f"oT{c}") for c in range(NCH)]

                def emit_pair(c2):
                    # Heads 2*c2 (dims at partitions 0:64) and 2*c2+1 (64:128)
                    # processed together: the dots matmuls of the two heads use
                    # disjoint PE row groups, so they execute CONCURRENTLY.
                    # Query span 0:1024 only; queries 1024:1026 are handled by
                    # the tails epilogue after all pairs (no PSUM left here).
                    h0, h1 = 2 * c2, 2 * c2 + 1
                    k0, q0 = krot[c2][0:64, :], qrot[c2][0:64, :]
                    k1, q1 = krot[c2][64:128, :], qrot[c2][64:128, :]
                    av0 = pavp.tile([128, NSP], F32, tag="av0")
                    av1 = pavp.tile([128, NSP], F32, tag="av1")
                    ats = {}
                    steps = [(j, qb) for j in range(TCH) for qb in range(2)]
                    DIST = int(os.environ.get("AVDIST", "4"))

                    def emit_av(s):
                        j, qb = steps[s]
                        jr, lo = KSZ[j], qb * 512
                        at = ats[(j, qb)]
                        nc.tensor.matmul(av0[0:65, lo:lo + 512], v_sb[j][:jr, h0, :],
                                         at[:jr, 0:512], start=(j == 0), stop=(j == TCH - 1))
                        nc.tensor.matmul(av1[0:65, lo:lo + 512], v_sb[j][:jr, h1, :],
                                         at[:jr, 512:1024], start=(j == 0), stop=(j == TCH - 1))

                    for s, (j, qb) in enumerate(steps):
                        jr, lo = KSZ[j], qb * 512
                        jc = slice(j * 128, j * 128 + jr)
                        pd = pbig.tile([128, NSP], F32, tag="pbig")
                        at = attnp.tile([128, NSP], F32R, tag="at")
                        nc.tensor.matmul(pd[:jr, 0:512], k0[:, jc], q0[:, lo:lo + 512],
                                         start=True, stop=True)
                        nc.tensor.matmul(pd[:jr, 512:1024], k1[:, jc], q1[:, lo:lo + 512],
                                         start=True, stop=True)
                        nc.scalar.activation(at[:jr, :], pd[:jr, :], AF.Exp, scale=SM)
                        ats[(j, qb)] = at
                        if s >= DIST:
                            emit_av(s - DIST)
                    for s in range(len(steps) - DIST, len(steps)):
                        emit_av(s)
                    # evacuate values + den rows to SBUF (DVE, ~1us each) so the
                    # av PSUM banks free fast; one batched reciprocal per pair.
                    dens = asmp.tile([2, NSP], F32, tag="dens")
                    avss = []
                    for idx, avp in ((0, av0), (1, av1)):
                        avs = asmp.tile([128, NSP], F32, tag=f"avs{idx}")
                        nc.vector.tensor_scalar(out=avs[0:64, :], in0=avp[0:64, :],
                                                scalar1=1.0, scalar2=None, op0=ALU.mult)
                        nc.vector.tensor_scalar(out=dens[idx:idx + 1, :],
                                                in0=avp[64:65, :],
                                                scalar1=1.0, scalar2=None, op0=ALU.mult)
                        avss.append(avs)
                    rden = asmp.tile([2, NSP], F32, tag="rden")
                    nc.vector.reciprocal(rden, dens)
                    for idx in range(2):
                        dbc = asmp.tile([64, NSP], F32, tag=f"dbc{idx}")
                        nc.gpsimd.partition_broadcast(dbc, rden[idx:idx + 1, :])
                        nc.vector.tensor_mul(oT[c2][64 * idx:64 * idx + 64, 0:NSP],
                                             avss[idx][0:64, :], dbc)

                if not _SKIP_ATTN:
                    for c2 in range(HEADS // 2):
                        emit_pair(c2)

        # ===== tails epilogue: queries 1024:1026 for all heads =====
        # (query 1024 is the last real token; 1025 is the pad, discarded)
        if not _SKIP_ATTN:
            with tc.tile_pool(name="ptl", bufs=1, space="PSUM") as ptl, \
                 tc.tile_pool(name="tsb", bufs=1) as tsb:
                tp = ptl.tile([128, 216], F32, tag="tp")
                nc.vector.memset(tp, 0.0)
                for j in range(TCH):
                    jr = KSZ[j]
                    jc = slice(j * 128, j * 128 + jr)
                    for c2 in range(HEADS // 2):
                        base = 24 * j + 4 * c2
                        nc.tensor.matmul(tp[:jr, base:base + 2],
                                         krot[c2][0:64, jc], qrot[c2][0:64, 1024:1026],
                                         start=True, stop=True)
                        nc.tensor.matmul(tp[:jr, base + 2:base + 4],
                                         krot[c2][64:128, jc], qrot[c2][64:128, 1024:1026],
                                         start=True, stop=True)
                att_t = tsb.tile([128, 216], F32R, tag="att_t")
                nc.scalar.activation(att_t, tp, AF.Exp, scale=SM)
                avt = ptl.tile([128, 24], F32, tag="avt")
                for c2 in range(HEADS // 2):
                    for hh in range(2):
                        cc = 4 * c2 + 2 * hh
                        for j in range(TCH):
                            jr = KSZ[j]
                            nc.tensor.matmul(avt[0:65, cc:cc + 2],
                                             v_sb[j][:jr, 2 * c2 + hh, :],
                                             att_t[:jr, 24 * j + cc:24 * j + cc + 2],
                                             start=(j == 0), stop=(j == TCH - 1))
                avts = tsb.tile([128, 24], F32, tag="avts")
                nc.vector.tensor_scalar(out=avts[0:65, :], in0=avt[0:65, :],
                                        scalar1=1.0, scalar2=None, op0=ALU.mult)
                rdent = tsb.tile([1, 24], F32, tag="rdent")
                nc.vector.reciprocal(rdent, avts[64:65, :])
                dbct = tsb.tile([64, 24], F32, tag="dbct")
                nc.gpsimd.partition_broadcast(dbct, rdent)
                for c2 in range(HEADS // 2):
                    for hh in range(2):
                        cc = 4 * c2 + 2 * hh
                        nc.vector.tensor_mul(oT[c2][64 * hh:64 * hh + 64, 1024:1026],
                                             avts[0:64, cc:cc + 2], dbct[:, cc:cc + 2])

        if _SKIP_ATTN:
            with tc.tile_pool(name="osp0", bufs=1) as osp0:
                z = osp0.tile([128, DIM], F32, tag="z")
                nc.vector.memset(z, 0.0)
                nc.sync.dma_start(out=dr["y"][0:128], in_=z)
            return

        # ============ Phase D: output projection ============
        if _SKIP_ATTN:
            # timing experiment: skip attention+output, keep a trivial y write
            zsb = asmp_dummy = None
            with tc.tile_pool(name="osp0", bufs=1) as osp0:
                z = osp0.tile([128, DIM], F32, tag="z")
                nc.vector.memset(z, 0.0)
                nc.sync.dma_start(out=dr["y"][0:128], in_=z)
            return
        with tc.tile_pool(name="wop", bufs=1) as wop, \
             tc.tile_pool(name="psop", bufs=2, space="PSUM") as psop, \
             tc.tile_pool(name="osp", bufs=3) as osp:
            wo_sb = []
            for c in range(NCH):
                w = wop.tile([128, DIM], F32R, tag=f"wo{c}", name=f"wo{c}")
                nc.sync.dma_start(out=w, in_=dr["wo"][c * 128:(c + 1) * 128])
                wo_sb.append(w)
            for t in range(TCH):
                rows = KSZ[t]
                ysz = YSZ[t]
                ps = psop.tile([128, DIM], F32, tag="pso")
                for c in range(NCH):
                    st, sp = c == 0, c == NCH - 1
                    lhs = oT[c][:, t * 128:t * 128 + rows]
                    nc.tensor.matmul(ps[:rows, 0:512], lhs, wo_sb[c][:, 0:512], start=st, stop=sp)
                    nc.tensor.matmul(ps[:rows, 512:768], lhs, wo_sb[c][:, 512:768], start=st, stop=sp)
                ot = osp.tile([128, DIM], F32, tag="ot")
                nc.vector.tensor_add(ot[:rows], ps[:rows], ob_sb[:rows])
                nc.sync.dma_start(out=dr["y"][t * 128:t * 128 + ysz], in_=ot[:ysz])


def _host_prep(inputs):
    """Build the per-core input maps from the full problem inputs."""
    x = np.asarray(inputs["x"], np.float32)
    sin = np.asarray(inputs["sin"], np.float32)
    cos = np.asarray(inputs["cos"], np.float32)
    ln_g = np.asarray(inputs["ln_g"], np.float32)
    ln_b = np.asarray(inputs["ln_b"], np.float32)
    dw_w = np.asarray(inputs["dw_w"], np.float32)
    pw_w = np.asarray(inputs["pw_w"], np.float32)
    kv_w = np.asarray(inputs["kv_w"], np.float32)
    out_w = np.asarray(inputs["out_w"], np.float32)
    out_b = np.asarray(inputs["out_b"], np.float32)

    wk = np.ascontiguousarray(kv_w[:, :DIM])
    wv = np.ascontiguousarray(kv_w[:, DIM:])
    wq = np.ascontiguousarray(pw_w[:, :, 0, 0].T)  # [in, out]
    wd = dw_w.reshape(DIM, 25)
    wd_arr = np.ascontiguousarray(wd)

    cosf = np.ones((128, NT2), np.float32)
    sinf = np.zeros((128, NT2), np.float32)
    for half in (0, 64):
        for d in range(32):
            cosf[half + d, 1:1025] = cos[:, d]
            sinf[half + d, 1:1025] = -sin[:, d] if d % 2 == 0 else sin[:, d]

    ident = np.eye(128, dtype=np.float32)
    ones12 = np.ones((128, HEADS), np.float32)
    ones12z = np.zeros((128, HEADS), np.float32)
    ones12z[0, :] = 1.0
    vzero = np.zeros((1, HEADS * (DH + 1)), np.float32)

    shared = dict(wk=wk, wv=wv, wq=wq, wo=out_w, wd=wd_arr, cosf=cosf, sinf=sinf,
                  ident=ident, ones12=ones12, ones12z=ones12z, vzero=vzero,
                  gvec=ln_g, bvec=ln_b, obias=out_b)
    in_maps = []
    for c in range(NCORES):
        m = dict(shared)
        m["x"] = np.ascontiguousarray(x[c])
        in_maps.append(m)
    return in_maps


_PROGRAM = None


def kernel(**inputs):
    global _PROGRAM
    if _PROGRAM is None:
        _PROGRAM = _build_program()
    in_maps = _host_prep(inputs)
    res = bass_utils.run_bass_kernel_spmd(_PROGRAM, in_maps, core_ids=list(range(NCORES)))
    return np.stack([res.results[c]["y"] for c in range(NCORES)]).astype(np.float32)

